# revision 46
# baseline (speedup 1.0000x reference)
"""Trainium2 Bass kernel for nn_Block_69861938036758 (sparse_attention).

Data-parallel over batch B=8 across 8 NeuronCores (one batch element per
core, no collectives). Per core, one fused transformer block:

  LN1 -> per-modality QKV -> masked per-modality softmax with
  modality-importance weighting -> AV -> proj -> residual -> LN2 ->
  fc1 -> exact gelu -> fc2 -> residual

Layout choices (matmul operands bf16, fp32 accumulation everywhere):
  - Scores computed transposed: S^T[ktok, qtok], so the softmax key-sums are
    matmul contractions and the prob matrix is never transposed.
  - Per-modality row-sums ride free as an extra ones-column in V.
  - Modality-importance means via the factorization
      mean[m,h] = SCALE/(nz_m*N) * <qsum_h, ksum_{m,h}>.
  - LN transposes on the DMA XBAR (dma_start_transpose), not the PE.
  - Per-query normalization: reciprocal on the [1,N] row-sum first, then a
    tiny DRAM bounce broadcasts the bf16 reciprocal across 64 partitions.
  - Engine balance: exp on ACT; PSUM evacuation on Pool (gpsimd); combines
    all-bf16 on DVE (2x mode); DMAs issued from SP (hwdge).
"""
import numpy as np
import ml_dtypes
from contextlib import ExitStack

import concourse.bass as bass
import concourse.bacc as bacc
import concourse.tile as tile
from concourse import mybir
from concourse.bass_utils import run_bass_kernel_spmd
from concourse.masks import make_identity

F32 = mybir.dt.float32
BF16 = mybir.dt.bfloat16
AF = mybir.ActivationFunctionType
ALU = mybir.AluOpType

SIZES = (256, 512, 256)
H, DIM, HD, N = 12, 768, 64, 1024
NT = N // 128            # 8 token tiles
SCALE = HD ** -0.5
MB = -30000.0            # additive mask bias (exp underflows to exact 0)
MOD_OFF = (0, 256, 768)
MOD_CHUNKS = ((0, 2), (2, 6), (6, 8))   # ktok 128-chunk ranges per modality
MT_MOD = (0, 0, 1, 1, 1, 1, 2, 2)       # modality of each 128-token tile
EPS = 1e-5

_CACHE = {}


def _emit(ctx, tc, nc, T, flags, sfx="", out_name="out"):
    apply_ln1, apply_ln2, use_pb, use_f1b, use_f2b, need_eps = flags
    v = nc.vector
    s = nc.scalar
    pe = nc.tensor
    g = nc.gpsimd

    const = ctx.enter_context(tc.tile_pool(name="const" + sfx, bufs=1))

    mb_sb = const.tile([128, 8], F32)
    maskrep = const.tile([128, N], F32)
    denr = const.tile([128, 3], F32)
    br12 = const.tile([12, 3], F32)
    seg_sb = const.tile([128, 6, 12], BF16)
    eps_t = const.tile([128, 1], F32)
    v.memset(eps_t[:], EPS)
    # broadcast modality weights; partition 64 holds 1.0 so a single
    # tensor_scalar evacuates U rows 0..63 scaled and the row-sum unscaled
    mult65 = const.tile([128, 12, 3], F32)
    v.memset(mult65[64:128, :, :], 1.0)

    def load_consts():
        nc.sync.dma_start(out=mb_sb[:], in_=T["mb2d"].ap())
        mf = T["maskf"].ap()
        nc.sync.dma_start(out=maskrep[:], in_=bass.AP(
            tensor=mf.tensor, offset=mf.offset, ap=[[0, 128], [1, N]]))
        d3 = T["den3"].ap()
        nc.sync.dma_start(out=denr[:], in_=bass.AP(
            tensor=d3.tensor, offset=d3.offset, ap=[[0, 128], [1, 3]]))
        b3 = T["bias3"].ap()
        nc.sync.dma_start(out=br12[:], in_=bass.AP(
            tensor=b3.tensor, offset=b3.offset, ap=[[0, 12], [1, 3]]))
        nc.sync.dma_start(out=seg_sb[:], in_=T["seg"].ap())

    def brep(name):
        t = const.tile([128, DIM], F32, tag=name)
        ap = T[name].ap()
        nc.sync.dma_start(out=t[:], in_=bass.AP(
            tensor=ap.tensor, offset=ap.offset, ap=[[0, 128], [1, DIM]]))
        return t

    ln1w_r = brep("ln1w") if apply_ln1 else None
    ln1b_r = brep("ln1b") if apply_ln1 else None
    ln2w_r = brep("ln2w") if apply_ln2 else None
    ln2b_r = brep("ln2b") if apply_ln2 else None
    pb_r = brep("pb") if use_pb else None
    f2b_r = brep("f2b") if use_f2b else None
    f1b_sb = None
    if use_f1b:
        f1b_sb = const.tile([128, 24], F32)
        nc.sync.dma_start(out=f1b_sb[:], in_=T["f1b"].ap())

    # manually-managed pools; stack (LIFO) allocator => nest lifetimes.
    p_x = tc.alloc_tile_pool(name="p_x" + sfx, bufs=1)          # [A..E]
    p_ot = tc.alloc_tile_pool(name="p_ot" + sfx, bufs=1)        # [A..E]
    p_qktv = tc.alloc_tile_pool(name="p_qktv" + sfx, bufs=1)    # [A..D]
    x_sb = [p_x.tile([128, 2, DIM], BF16, tag=f"x{sg}", name=f"x{sg}")
            for sg in range(4)]
    OT = [p_ot.tile([128, N], BF16, tag=f"ot{pc}", name=f"ot{pc}")
          for pc in range(6)]
    QT = p_qktv.tile([128, 6, N], BF16)
    KT = p_qktv.tile([128, 6, N], BF16)
    V = p_qktv.tile([128, NT, H, HD + 1], BF16)

    v.memset(V[:, :, :, HD:HD + 1], 1.0)  # ones column -> free row-sums

    def layer_norm_into(pool, src_ap, wr, br_, tag):
        """LN over free dim (768) -> bf16 [128, 768] tile."""
        stats = pool.tile([128, 3, 6], F32, tag=tag + "_st")
        for i in range(3):
            v.bn_stats(out=stats[:, i, :], in_=src_ap[:, i * 256:(i + 1) * 256])
        mv = pool.tile([128, 2], F32, tag=tag + "_mv")
        v.bn_aggr(out=mv[:], in_=stats[:])
        sd = pool.tile([128, 1], F32, tag=tag + "_sd")
        s.activation(sd[:], mv[:, 1:2], AF.Sqrt, bias=eps_t[:])
        rstd = pool.tile([128, 1], F32, tag=tag + "_rs")
        v.reciprocal(rstd[:], sd[:])
        out_bf = pool.tile([128, DIM], BF16, tag=tag + "_o")
        if wr is None:
            v.tensor_scalar(out_bf[:], src_ap, mv[:, 0:1], rstd[:],
                            op0=ALU.subtract, op1=ALU.mult)
        else:
            tmp = pool.tile([128, DIM], F32, tag=tag + "_t")
            v.tensor_scalar(tmp[:], src_ap, mv[:, 0:1], rstd[:],
                            op0=ALU.subtract, op1=ALU.mult)
            v.tensor_tensor(tmp[:], tmp[:], wr[:], op=ALU.mult)
            v.tensor_tensor(out_bf[:], tmp[:], br_[:], op=ALU.add)
        return out_bf

    # ---- Phase A: LN1 + DMA-transpose to xnT; Phase B: QKV + stats ----------
    stat = tc.alloc_tile_pool(name="stat" + sfx, bufs=1)
    with tc.tile_pool(name="qkvw" + sfx, bufs=1) as qkvw:

        wq_sb = [qkvw.tile([128, 6, 6, 128], BF16, tag=f"wq{m}", name=f"wq{m}")
                 for m in range(3)]
        wk_sb = [qkvw.tile([128, 6, 6, 128], BF16, tag=f"wk{m}", name=f"wk{m}")
                 for m in range(3)]
        wv_sb = qkvw.tile([128, 3, 6, DIM], BF16)
        # SP DMA order: x tiles first (unblocks LN), then wq per modality
        # (unblocks Q), then wk, wv, then the small constants
        for mt in range(NT):
            nc.sync.dma_start(out=x_sb[mt // 2][:, mt % 2, :],
                              in_=T["x"].ap()[:, mt])
        for m in range(3):
            nc.sync.dma_start(out=wq_sb[m][:], in_=T["wq"].ap()[:, m])
        for m in range(3):
            nc.sync.dma_start(out=wk_sb[m][:], in_=T["wk"].ap()[:, m])
        for m in range(3):
            nc.sync.dma_start(out=wv_sb[:, m, :, :], in_=T["wv"].ap()[:, m])
        load_consts()
        # per-seg xnT tiles so Q matmuls start as soon as their two token
        # tiles are transposed
        xnT = [qkvw.tile([128, 6, 256], BF16, tag=f"xnT{sg}", name=f"xnT{sg}")
               for sg in range(4)]

        with tc.tile_pool(name="lnp" + sfx, bufs=3) as lnp:
            for mt in range(NT):
                xn = layer_norm_into(lnp, x_sb[mt // 2][:, mt % 2, :],
                                     ln1w_r, ln1b_r, "ln1")
                for kc in range(6):
                    s.dma_start_transpose(
                        xnT[mt // 2][:, kc, (mt % 2) * 128:(mt % 2 + 1) * 128],
                        xn[:, kc * 128:(kc + 1) * 128])

        # all-Q then all-K matmuls (K weights arrive later); per-pc modality
        # stats interleaved on DVE behind the K matmuls
        qs = stat.tile([128, 6], F32)
        ks = stat.tile([128, 6, 3], F32)
        with tc.tile_pool(name="pqk" + sfx, bufs=2, space="PSUM") as pqk, \
             tc.tile_pool(name="statw" + sfx, bufs=2) as statw:
            for w_sb, dst, do_stats in ((wq_sb, QT, False), (wk_sb, KT, True)):
                for pc in range(6):
                    ps = pqk.tile([128, N], F32, tag="q")
                    for seg, m in ((0, 0), (1, 1), (2, 1), (3, 2)):
                        o = seg * 256
                        for kc in range(6):
                            pe.matmul(ps[:, o:o + 256], w_sb[m][:, kc, pc, :],
                                      xnT[seg][:, kc, :],
                                      start=(kc == 0), stop=(kc == 5))
                    s.copy(dst[:, pc, :], ps[:])
                    if do_stats:
                        v.reduce_sum(qs[:, pc:pc + 1], QT[:, pc, :],
                                     axis=mybir.AxisListType.X)
                        km = statw.tile([128, N], F32, tag="km")
                        v.tensor_tensor(km[:], KT[:, pc, :], maskrep[:],
                                        op=ALU.mult)
                        for m in range(3):
                            o, sz = MOD_OFF[m], SIZES[m]
                            v.reduce_sum(ks[:, pc, m:m + 1], km[:, o:o + sz],
                                         axis=mybir.AxisListType.X)

        # modality-importance means + mult (overlaps the V matmuls below)
        with tc.tile_pool(name="pv" + sfx, bufs=2, space="PSUM") as pv, \
             tc.tile_pool(name="pmn" + sfx, bufs=1, space="PSUM") as pmn:
            for m in range(3):
                v.tensor_scalar(ks[:, :, m], ks[:, :, m], denr[:, m:m + 1],
                                SCALE, op0=ALU.mult, op1=ALU.mult)
            prod = stat.tile([128, 6, 3], BF16)
            for pc in range(6):
                v.tensor_scalar(prod[:, pc, :], ks[:, pc, :], qs[:, pc:pc + 1],
                                None, op0=ALU.mult)
            pm = pmn.tile([12, 3], F32)
            for pc in range(6):
                pe.matmul(pm[:], seg_sb[:, pc, :], prod[:, pc, :],
                          start=(pc == 0), stop=(pc == 5))
            mn = stat.tile([12, 3], F32)
            v.tensor_tensor(mn[:], pm[:], br12[:], op=ALU.add)
            me = stat.tile([12, 3], F32)
            msum = stat.tile([12, 1], F32)
            s.activation(me[:], mn[:], AF.Exp, accum_out=msum[:])
            mrec = stat.tile([12, 1], F32)
            v.reciprocal(mrec[:], msum[:])
            mult_sb = stat.tile([12, 3], F32)
            v.tensor_scalar(mult_sb[:], me[:], mrec[:], None, op0=ALU.mult)
            drs = ctx.enter_context(
                tc.tile_pool(name="drs" + sfx, bufs=4, space="DRAM"))
            scm = drs.tile([12, 3], F32, tag="scm")
            nc.sync.dma_start(out=scm[:], in_=mult_sb[:])
            sc = scm[:]
            nc.sync.dma_start(out=mult65[0:64, :, :], in_=bass.AP(
                tensor=sc.tensor, offset=sc.offset, ap=[[0, 64], [3, 12], [1, 3]]))

            # V matmuls keep the PE busy while the mean-chain completes
            for mt in range(NT):
                m = MT_MOD[mt]
                ps = pv.tile([128, DIM], F32, tag="v")
                for fo, fs in ((0, 512), (512, 256)):
                    for kc in range(6):
                        pe.matmul(ps[:, fo:fo + fs],
                                  xnT[mt // 2][:, kc,
                                               (mt % 2) * 128:(mt % 2 + 1) * 128],
                                  wv_sb[:, m, kc, fo:fo + fs],
                                  start=(kc == 0), stop=(kc == 5))
                s.copy(V[:, mt, :, 0:HD],
                       ps[:].rearrange("p (h d) -> p h d", h=H))

    stat.release()

    # ---- MLP weights: loaded piecewise between phase-D bounce DMAs so the
    # prefetch never monopolizes the DMA engines ahead of the softmax chain.
    p_mlpw = tc.alloc_tile_pool(name="p_mlpw" + sfx, bufs=1, side="right")
    f1_sb = p_mlpw.tile([128, 6, 4 * DIM], BF16)
    f2_sb = p_mlpw.tile([128, 24, DIM], BF16)
    pw_sb = p_mlpw.tile([128, 6, DIM], BF16)

    def _prefetch_pieces():
        for pc in range(0, 6, 3):
            yield pw_sb[:, pc:pc + 3, :], T["pw"].ap()[:, pc:pc + 3]
        for kc in range(6):
            for hf in range(2):
                o = hf * 2 * DIM
                yield (f1_sb[:, kc, o:o + 2 * DIM],
                       T["f1"].ap()[:, kc, o:o + 2 * DIM])
        for oc in range(0, 24, 2):
            yield f2_sb[:, oc:oc + 2, :], T["f2"].ap()[:, oc:oc + 2]
    _pieces = _prefetch_pieces()

    # ---- Phase D: attention (transposed scores) -----------------------------
    # Per (h, m): score chunks -> exp(ACT) -> AV accumulate (PE). Pool (gpsimd)
    # evacuates U[0:64] to bf16 SBUF with the modality weight folded in via the
    # f32 scalar operand, and the raw row-sum row to bf16; a tiny DRAM bounce
    # broadcasts the row across 64 partitions; the combine is 3 divides + 2
    # adds, all-bf16 on DVE (2x mode).
    with tc.tile_pool(name="pst" + sfx, bufs=2, space="PSUM") as pst, \
         tc.tile_pool(name="pu" + sfx, bufs=2, space="PSUM") as pu, \
         tc.tile_pool(name="ep" + sfx, bufs=4) as ep, \
         tc.tile_pool(name="usb" + sfx, bufs=7) as usb, \
         tc.tile_pool(name="rrp" + sfx, bufs=3) as rrp, \
         tc.tile_pool(name="rp" + sfx, bufs=3) as rp, \
         tc.tile_pool(name="cp" + sfx, bufs=1) as cp:
        def emit_combine(h, Us, Rm3):
            po = (h % 2) * 64
            pc = h // 2
            with nc.allow_low_precision(reason="bf16 attn combine"):
                acc = cp.tile([64, N], BF16, tag="acc", name=f"acc_{h}")
                v.tensor_tensor(acc[:], Us[0][0:HD, :], Rm3[:, 0, :], op=ALU.mult)
                t1 = cp.tile([64, N], BF16, tag="t1", name=f"t1_{h}")
                v.tensor_tensor(t1[:], Us[1][0:HD, :], Rm3[:, 1, :], op=ALU.mult)
                g.tensor_tensor(acc[:], acc[:], t1[:], op=ALU.add)
                t2 = cp.tile([64, N], BF16, tag="t2", name=f"t2_{h}")
                v.tensor_tensor(t2[:], Us[2][0:HD, :], Rm3[:, 2, :], op=ALU.mult)
                v.tensor_tensor(OT[pc][po:po + 64, :], acc[:], t2[:],
                                op=ALU.add)

        CHUNK_MOD = (0, 0, 1, 1, 1, 1, 2, 2)
        for h in range(H):
            po = (h % 2) * 64
            pc = h // 2
            Us = []
            rows3 = drs.tile([3, N], BF16, tag="rows3", name=f"rows3_{h}")
            # chunk-level software pipeline: AV lags the score/exp by one
            # chunk so the in-order PE never waits on the exp.
            Es = [None] * NT
            Um = None
            for ci in range(NT + 1):
                if ci < NT:
                    st = pst.tile([128, N], F32, tag="st", name=f"st_{h}_{ci}")
                    for half in range(2):
                        hs = slice(half * 512, (half + 1) * 512)
                        pe.matmul(st[:, hs],
                                  KT[po:po + 64, pc, ci * 128:(ci + 1) * 128],
                                  QT[po:po + 64, pc, hs], start=True, stop=True)
                    E = ep.tile([128, N], BF16, tag="e", name=f"e_{h}_{ci}")
                    s.activation(E[:], st[:], AF.Exp,
                                 bias=mb_sb[:, ci:ci + 1], scale=SCALE)
                    Es[ci] = E
                if ci >= 1:
                    c = ci - 1
                    m = CHUNK_MOD[c]
                    c0, c1 = MOD_CHUNKS[m]
                    if c == c0:
                        Um = pu.tile([HD + 1, N], F32, tag="u",
                                     name=f"u_{h}_{m}")
                    for half in range(2):
                        hs = slice(half * 512, (half + 1) * 512)
                        pe.matmul(Um[:, hs], V[:, c, h, :], Es[c][:, hs],
                                  start=(c == c0), stop=(c == c1 - 1))
                    if c == c1 - 1:
                        Usb = usb.tile([HD + 1, N], BF16, tag="usb",
                                       name=f"usb_{h}_{m}")
                        with nc.allow_low_precision(reason="bf16 attn path"):
                            if need_eps:
                                v.tensor_scalar(Usb[0:HD, :], Um[0:HD, :],
                                                mult65[0:HD, h, m:m + 1], None,
                                                op0=ALU.mult)
                                v.tensor_scalar(Usb[HD:HD + 1, :],
                                                Um[HD:HD + 1, :],
                                                1e-12, None, op0=ALU.add)
                            else:
                                v.tensor_scalar(Usb[:], Um[:],
                                                mult65[0:HD + 1, h, m:m + 1],
                                                None, op0=ALU.mult)
                        nc.sync.dma_start(out=rows3[m:m + 1, :],
                                          in_=Usb[HD:HD + 1, :])
                        piece = next(_pieces, None)
                        if piece is not None:
                            nc.sync.dma_start(out=piece[0], in_=piece[1])
                        Us.append(Usb)
            if h < H - 1:
                # batched per-head row-sum reciprocal: spread the 3 rows over
                # 64 partitions (48 elems each), reciprocal there, write back,
                # then one stride-0 broadcast of all 3 reciprocal rows.
                r3 = rows3[:]
                rsp = rrp.tile([64, 3 * N // 64], BF16, tag="rr",
                               name=f"rr_{h}")
                nc.sync.dma_start(out=rsp[:], in_=bass.AP(
                    tensor=r3.tensor, offset=r3.offset,
                    ap=[[3 * N // 64, 64], [1, 3 * N // 64]]))
                with nc.allow_low_precision(reason="bf16 softmax denom"):
                    v.reciprocal(rsp[:], rsp[:])
                scr3 = drs.tile([3, N], BF16, tag="scr3", name=f"scr3_{h}")
                sa = scr3[:]
                nc.sync.dma_start(out=bass.AP(
                    tensor=sa.tensor, offset=sa.offset,
                    ap=[[3 * N // 64, 64], [1, 3 * N // 64]]), in_=rsp[:])
                Rm3 = rp.tile([64, 3, N], BF16, tag="rm", name=f"rm_{h}")
                nc.sync.dma_start(out=Rm3[:], in_=bass.AP(
                    tensor=sa.tensor, offset=sa.offset,
                    ap=[[0, 64], [N, 3], [1, N]]))
                emit_combine(h, Us, Rm3)
            else:
                # last head: shortest-latency path — broadcast the raw rows
                # immediately and fold mult+reciprocal into stt combines so
                # proj isn't gated on a long bounce chain.
                r3 = rows3[:]
                Rm3 = rp.tile([64, 3, N], BF16, tag="rm", name=f"rm_{h}")
                for m in range(3):
                    nc.sync.dma_start(out=Rm3[:, m, :], in_=bass.AP(
                        tensor=r3.tensor, offset=r3.offset + m * N,
                        ap=[[0, 64], [1, N]]))
                with nc.allow_low_precision(reason="bf16 attn combine"):
                    Rc = cp.tile([64, 3, N], BF16, tag="rc", name=f"rc_{h}")
                    v.reciprocal(Rc[:], Rm3[:])
                    acc = cp.tile([64, N], BF16, tag="acc", name=f"acc_{h}")
                    v.tensor_tensor(acc[:], Us[0][0:HD, :], Rc[:, 0, :],
                                    op=ALU.mult)
                    t1 = cp.tile([64, N], BF16, tag="t1", name=f"t1_{h}")
                    v.tensor_tensor(t1[:], Us[1][0:HD, :], Rc[:, 1, :],
                                    op=ALU.mult)
                    g.tensor_tensor(acc[:], acc[:], t1[:], op=ALU.add)
                    t2 = cp.tile([64, N], BF16, tag="t2", name=f"t2_{h}")
                    v.tensor_tensor(t2[:], Us[2][0:HD, :], Rc[:, 2, :],
                                    op=ALU.mult)
                    v.tensor_tensor(OT[pc][po:po + 64, :], acc[:], t2[:],
                                    op=ALU.add)

    p_qktv.release()

    # ---- Phases E..H fused --------------------------------------------------
    # Per token-half: proj + residual + LN2 + hT transpose for its 4 tiles,
    # then fc1+gelu for that half (PE busy while the other half's LN2 chain
    # runs on DVE/ACT). fc2 + residual + output DMA at the end.
    p_x2h = tc.alloc_tile_pool(name="p_x2h" + sfx, bufs=1, side="right")
    x2_sb = p_x2h.tile([128, NT, DIM], F32)
    hT = [p_x2h.tile([128, 6, 512], BF16, tag=f"hT{hf}", name=f"hT{hf}")
          for hf in range(2)]
    gT = [p_x2h.tile([128, 24, 512], BF16, tag=f"gT{hf}", name=f"gT{hf}")
          for hf in range(2)]
    with tc.tile_pool(name="py" + sfx, bufs=2, space="PSUM") as py, \
         tc.tile_pool(name="pg" + sfx, bufs=2, space="PSUM") as pg, \
         tc.tile_pool(name="pz" + sfx, bufs=1, space="PSUM") as pz, \
         tc.tile_pool(name="lnp2" + sfx, bufs=3) as lnp2, \
         tc.tile_pool(name="op" + sfx, bufs=2) as op:
        for hf in range(2):
            for mt in range(hf * 4, hf * 4 + 4):
                ps = py.tile([128, DIM], F32, tag="y")
                for fo, fs in ((0, 512), (512, 256)):
                    for pc in range(6):
                        pe.matmul(ps[:, fo:fo + fs],
                                  OT[pc][:, mt * 128:(mt + 1) * 128],
                                  pw_sb[:, pc, fo:fo + fs],
                                  start=(pc == 0), stop=(pc == 5))
                if use_pb:
                    v.tensor_tensor(ps[:], ps[:], pb_r[:], op=ALU.add)
                v.tensor_tensor(x2_sb[:, mt, :], ps[:],
                                x_sb[mt // 2][:, mt % 2, :], op=ALU.add)
                hn = layer_norm_into(lnp2, x2_sb[:, mt, :], ln2w_r, ln2b_r,
                                     "ln2")
                for kc in range(6):
                    s.dma_start_transpose(
                        hT[hf][:, kc, (mt % 4) * 128:(mt % 4 + 1) * 128],
                        hn[:, kc * 128:(kc + 1) * 128])
            for oc in range(24):
                ps = pg.tile([128, 512], F32, tag="g")
                for kc in range(6):
                    pe.matmul(ps[:], f1_sb[:, kc, oc * 128:(oc + 1) * 128],
                              hT[hf][:, kc, :], start=(kc == 0), stop=(kc == 5))
                if use_f1b:
                    s.activation(gT[hf][:, oc, :], ps[:], AF.Gelu,
                                 bias=f1b_sb[:, oc:oc + 1])
                else:
                    s.activation(gT[hf][:, oc, :], ps[:], AF.Gelu)
        for t in range(NT):
            ps = pz.tile([128, DIM], F32, tag="z")
            for fo, fs in ((0, 512), (512, 256)):
                for oc in range(24):
                    pe.matmul(ps[:, fo:fo + fs],
                              gT[t // 4][:, oc, (t % 4) * 128:(t % 4 + 1) * 128],
                              f2_sb[:, oc, fo:fo + fs],
                              start=(oc == 0), stop=(oc == 23))
            ob = op.tile([128, DIM], F32, tag="ob")
            if use_f2b:
                v.tensor_tensor(ob[:], ps[:], f2b_r[:], op=ALU.add)
                v.tensor_tensor(ob[:], ob[:], x2_sb[:, t, :], op=ALU.add)
            else:
                v.tensor_tensor(ob[:], ps[:], x2_sb[:, t, :], op=ALU.add)
            nc.sync.dma_start(out=T[out_name].ap()[t], in_=ob[:])

    p_ot.release()
    p_x.release()
    p_x2h.release()
    p_mlpw.release()


def _build(flags, reps=1):
    nc = bacc.Bacc("TRN2", target_bir_lowering=False, debug=False, num_devices=8)
    apply_ln1, apply_ln2, use_pb, use_f1b, use_f2b, need_eps = flags
    T = {}
    T["x"] = nc.dram_tensor("x", (128, NT, DIM), BF16, kind="ExternalInput")
    T["maskf"] = nc.dram_tensor("maskf", (N,), F32, kind="ExternalInput")
    T["mb2d"] = nc.dram_tensor("mb2d", (128, 8), F32, kind="ExternalInput")
    T["den3"] = nc.dram_tensor("den3", (3,), F32, kind="ExternalInput")
    T["bias3"] = nc.dram_tensor("bias3", (3,), F32, kind="ExternalInput")
    T["seg"] = nc.dram_tensor("seg", (128, 6, 12), BF16, kind="ExternalInput")
    T["wq"] = nc.dram_tensor("wq", (128, 3, 6, 6, 128), BF16, kind="ExternalInput")
    T["wk"] = nc.dram_tensor("wk", (128, 3, 6, 6, 128), BF16, kind="ExternalInput")
    T["wv"] = nc.dram_tensor("wv", (128, 3, 6, DIM), BF16, kind="ExternalInput")
    T["pw"] = nc.dram_tensor("pw", (128, 6, DIM), BF16, kind="ExternalInput")
    T["f1"] = nc.dram_tensor("f1", (128, 6, 4 * DIM), BF16, kind="ExternalInput")
    T["f2"] = nc.dram_tensor("f2", (128, 24, DIM), BF16, kind="ExternalInput")
    if apply_ln1:
        T["ln1w"] = nc.dram_tensor("ln1w", (DIM,), F32, kind="ExternalInput")
        T["ln1b"] = nc.dram_tensor("ln1b", (DIM,), F32, kind="ExternalInput")
    if apply_ln2:
        T["ln2w"] = nc.dram_tensor("ln2w", (DIM,), F32, kind="ExternalInput")
        T["ln2b"] = nc.dram_tensor("ln2b", (DIM,), F32, kind="ExternalInput")
    if use_pb:
        T["pb"] = nc.dram_tensor("pb", (DIM,), F32, kind="ExternalInput")
    if use_f1b:
        T["f1b"] = nc.dram_tensor("f1b", (128, 24), F32, kind="ExternalInput")
    if use_f2b:
        T["f2b"] = nc.dram_tensor("f2b", (DIM,), F32, kind="ExternalInput")
    for r in range(reps):
        T[f"out{r}"] = nc.dram_tensor(f"out{r}", (NT, 128, DIM), F32,
                                      kind="ExternalOutput")

    with tile.TileContext(nc) as tc:
        for r in range(reps):
            with ExitStack() as ctx:
                _emit(ctx, tc, nc, T, flags, sfx=f"_{r}", out_name=f"out{r}")
    nc.compile()
    return nc


def get_program(flags, reps=1):
    key = (flags, reps)
    if key not in _CACHE:
        _CACHE[key] = _build(flags, reps)
    return _CACHE[key]


def _bf(a):
    return np.ascontiguousarray(a, dtype=np.float32).astype(ml_dtypes.bfloat16)


def prepare(inputs):
    """Host-side prep: flags + per-core input maps."""
    x = np.asarray(inputs["x"], np.float32)
    mask = np.asarray(inputs["attention_mask"])
    ln1_w = np.asarray(inputs["ln1_w"], np.float32)
    ln1_b = np.asarray(inputs["ln1_b"], np.float32)
    ln2_w = np.asarray(inputs["ln2_w"], np.float32)
    ln2_b = np.asarray(inputs["ln2_b"], np.float32)
    proj_b = np.asarray(inputs["proj_b"], np.float32)
    fc1_b = np.asarray(inputs["fc1_b"], np.float32)
    fc2_b = np.asarray(inputs["fc2_b"], np.float32)
    qkv_ws = [np.asarray(inputs[k], np.float32)
              for k in ("qkv_text_w", "qkv_video_w", "qkv_audio_w")]
    proj_w = np.asarray(inputs["proj_w"], np.float32)
    fc1_w = np.asarray(inputs["fc1_w"], np.float32)
    fc2_w = np.asarray(inputs["fc2_w"], np.float32)

    nz_all = []
    for b in range(x.shape[0]):
        mf = (mask[b] != 0)
        nz_all.append([mf[o:o + sz].sum() for o, sz in zip(MOD_OFF, SIZES)])
    flags = (
        not (np.all(ln1_w == 1.0) and np.all(ln1_b == 0.0)),
        not (np.all(ln2_w == 1.0) and np.all(ln2_b == 0.0)),
        bool(np.any(proj_b != 0.0)),
        bool(np.any(fc1_b != 0.0)),
        bool(np.any(fc2_b != 0.0)),
        bool(np.any(np.array(nz_all) == 0)),
    )
    apply_ln1, apply_ln2, use_pb, use_f1b, use_f2b, need_eps = flags

    # shared (identical per core) tensors, DMA-friendly partition-first layouts
    def pack_qk(rows):
        # (3 modalities, 768 out, 768 in) -> [128 p(featin), 3, 6 kc, 6 pc, 128]
        a = np.stack([w[rows[0]:rows[1]].T.reshape(6, 128, 6, 128)
                      for w in qkv_ws])            # (3, kc, p, pc, j)
        return _bf(a.transpose(2, 0, 1, 3, 4))

    wq = pack_qk((0, 768))
    wk = pack_qk((768, 1536))
    wv = _bf(np.stack([w[1536:2304].T.reshape(6, 128, DIM) for w in qkv_ws])
             .transpose(2, 0, 1, 3))               # [128, 3, 6, 768]
    pw = _bf(proj_w.T.reshape(6, 128, DIM).transpose(1, 0, 2))
    f1 = _bf(fc1_w.T.reshape(6, 128, 4 * DIM).transpose(1, 0, 2))
    f2 = _bf(fc2_w.T.reshape(24, 128, DIM).transpose(1, 0, 2))
    seg = np.zeros((128, 6, 12), np.float32)
    for pc in range(6):
        seg[0:64, pc, 2 * pc] = 1.0
        seg[64:128, pc, 2 * pc + 1] = 1.0
    shared = {"wq": wq, "wk": wk, "wv": wv, "pw": pw, "f1": f1, "f2": f2,
              "seg": _bf(seg)}
    if apply_ln1:
        shared["ln1w"], shared["ln1b"] = ln1_w, ln1_b
    if apply_ln2:
        shared["ln2w"], shared["ln2b"] = ln2_w, ln2_b
    if use_pb:
        shared["pb"] = proj_b
    if use_f1b:
        shared["f1b"] = np.ascontiguousarray(fc1_b.reshape(24, 128).T)
    if use_f2b:
        shared["f2b"] = fc2_b

    in_maps = []
    for b in range(x.shape[0]):
        maskf = (mask[b] != 0).astype(np.float32)
        nz = np.array([maskf[o:o + sz].sum() for o, sz in zip(MOD_OFF, SIZES)],
                      np.float64)
        m = dict(shared)
        m["x"] = _bf(x[b].reshape(NT, 128, DIM).transpose(1, 0, 2))
        m["maskf"] = maskf
        m["mb2d"] = np.ascontiguousarray((MB * (1.0 - maskf)).reshape(8, 128).T)
        m["den3"] = np.where(nz > 0, 1.0 / np.maximum(nz * N, 1.0), 0.0).astype(np.float32)
        m["bias3"] = np.where(nz > 0, 0.0, MB).astype(np.float32)
        in_maps.append(m)
    return flags, in_maps


def kernel(**inputs):
    flags, in_maps = prepare(inputs)
    nc = get_program(flags)
    res = run_bass_kernel_spmd(nc, in_maps, list(range(len(in_maps))))
    out = np.stack([r["out0"].reshape(N, DIM) for r in res.results])
    return np.ascontiguousarray(out, dtype=np.float32)
